# revision 1
# baseline (speedup 1.0000x reference)
"""Two-layer GCN (PyG GCNConv semantics) on 8 Trainium2 NeuronCores.

Strategy (1D graph partitioning, destination-sharded):
  * Nodes are sorted by in-degree (descending), padded to a multiple of
    128*8, and chunked into groups of 128.  Group g is owned by core g%8.
    Node identity on device = "table row" t = k*(J*128) + p*J + j for core
    k, partition slot p, local group j.
  * All per-edge index work happens on the host: each destination node
    gets Dhat_j padded edge slots; slot (p, d) of a group holds the edge
    weight w_e and the table row of the source node.  Padding slots have
    w=0 and point at row 0.
  * On device, per layer:  gather source rows with indirect DMA from a
    DRAM table (f32 rows, 256B descriptors), multiply by
    w~ = w * dinv[dst] (broadcast over features), and reduce over the
    edge-slot axis with a strided DVE reduction.  Aggregation runs before
    the 64x64 weight matmul ((A x) W == A (x W)), so only J tiles need the
    transpose + matmul.  dinv[src] is folded into the gather table
    (x' = dinv * x), recomputed per layer; dinv[dst] is folded into w~.
  * The table for layer l+1 is built with one 8-core AllGather of the
    dinv-scaled layer output.
"""

import math
import sys
from contextlib import ExitStack

import numpy as np

if "/opt/trn_rl_repo" not in sys.path:
    sys.path.insert(0, "/opt/trn_rl_repo")

P = 128  # SBUF partitions
C = 8    # NeuronCores
F = 64   # feature width (in = hidden = out = 64)
GATHER_SLOT_BUDGET = 64  # max padded edge slots per gather batch (per partition)
WAVE = 8                 # groups per transform wave (8*64 = 512 = one PSUM bank)


# ---------------------------------------------------------------------------
# Host-side graph preprocessing (integer index work + permutations only)
# ---------------------------------------------------------------------------

def _plan(n_nodes, edge_index, edge_feats):
    N = int(n_nodes)
    G0 = math.ceil(N / P)
    G_total = math.ceil(G0 / C) * C
    J = G_total // C
    N_pad = G_total * P

    row = np.asarray(edge_index[0], dtype=np.int64)
    col = np.asarray(edge_index[1], dtype=np.int64)
    w = np.asarray(edge_feats, dtype=np.float32)

    # Self-loops are NOT materialized as edge slots: the self contribution
    # dinv[v]^2 * x[v] is added on-device from the SBUF-resident slice.
    r_all = row
    c_all = col
    w_all = w

    degc = np.bincount(c_all, minlength=N_pad)  # real in-degree (may be 0)
    order = np.argsort(-degc, kind="stable")    # descending degree
    s_of = np.empty(N_pad, np.int64)
    s_of[order] = np.arange(N_pad)
    g_of = s_of // P
    p_of = s_of % P
    k_of = g_of % C
    j_of = g_of // C
    t_of = k_of * (P * J) + p_of * J + j_of     # table row per node

    # per-group max degree; descending order => stripe max is the first one
    Dg = degc[order[np.arange(G_total) * P]]
    Dhat = Dg[0::C].astype(np.int64)  # [J], may be 0 for the tail
    off = np.concatenate([[0], np.cumsum(Dhat)]).astype(np.int64)
    SD = int(off[-1])

    # edge slot assignment: sort edges by destination table row
    tdst = t_of[c_all]
    oE = np.argsort(tdst, kind="stable")
    td = tdst[oE]
    dslot = np.arange(len(td), dtype=np.int64) - np.searchsorted(td, td, side="left")
    kk = td // (P * J)
    rem = td - kk * (P * J)
    pp = rem // J
    jj = rem - pp * J
    assert np.all(dslot < Dhat[jj]), "edge slot exceeded padded degree"

    w_pad = np.zeros((C, P, SD), np.float32)
    idx = np.zeros((C, P, SD), np.int32)
    colpos = off[jj] + dslot
    w_pad[kk, pp, colpos] = w_all[oE]
    idx[kk, pp, colpos] = t_of[r_all[oE]].astype(np.int32)

    # gather batches: consecutive groups packed to <= GATHER_SLOT_BUDGET slots
    batches = []  # (j0, j1, off0, off1)
    j0 = 0
    while j0 < J:
        j1 = j0 + 1
        while j1 < J and off[j1 + 1] - off[j0] <= GATHER_SLOT_BUDGET:
            j1 += 1
        if off[j1] > off[j0]:  # skip fully-empty tails
            batches.append((j0, j1, int(off[j0]), int(off[j1])))
        j0 = j1

    return dict(N=N, N_pad=N_pad, J=J, SD=SD, Dhat=Dhat, off=off, t_of=t_of,
                w_pad=w_pad, idx=idx, batches=batches)


def _shard_x(node_feats, plan):
    N, N_pad, J = plan["N"], plan["N_pad"], plan["J"]
    x_perm = np.zeros((N_pad, F), np.float32)
    x_perm[plan["t_of"][:N]] = np.asarray(node_feats, dtype=np.float32)
    # table row t = k*(P*J) + p*J + j  ->  [C, P, J*F]
    return x_perm.reshape(C, P, J, F).reshape(C, P, J * F)


# ---------------------------------------------------------------------------
# Device program
# ---------------------------------------------------------------------------

def _build(plan):
    from concourse import bacc, bass, mybir
    import concourse.tile as tile
    from concourse.masks import make_identity

    f32 = mybir.dt.float32
    i32 = mybir.dt.int32
    J, SD = plan["J"], plan["SD"]
    Dhat, off, batches = plan["Dhat"], plan["off"], plan["batches"]
    JP = J * P
    maxS = max(o1 - o0 for (_, _, o0, o1) in batches)

    nc = bacc.Bacc(None, target_bir_lowering=False, num_devices=C)

    x_in = nc.dram_tensor("x_slice", [P, J * F], f32, kind="ExternalInput")
    w_in = nc.dram_tensor("w_pad", [P, SD], f32, kind="ExternalInput")
    idx_in = nc.dram_tensor("idx", [P, SD], i32, kind="ExternalInput")
    W1_in = nc.dram_tensor("W1", [F, F], f32, kind="ExternalInput")
    W2_in = nc.dram_tensor("W2", [F, F], f32, kind="ExternalInput")
    b1_in = nc.dram_tensor("b1", [P, F], f32, kind="ExternalInput")
    b2_in = nc.dram_tensor("b2", [P, F], f32, kind="ExternalInput")
    out_t = nc.dram_tensor("out", [P, J * F], f32, kind="ExternalOutput")

    ag1 = nc.dram_tensor("ag_in1", [JP, F], f32)
    ag2 = nc.dram_tensor("ag_in2", [JP, F], f32)
    table1 = nc.dram_tensor("table1", [C * JP, F], f32)
    table2 = nc.dram_tensor("table2", [C * JP, F], f32)

    groups = [list(range(C))]

    with ExitStack() as ctx:
        tc = ctx.enter_context(tile.TileContext(nc))
        big = ctx.enter_context(tc.tile_pool(name="big", bufs=1))
        gp = ctx.enter_context(tc.tile_pool(name="gp", bufs=4))
        aT = ctx.enter_context(tc.tile_pool(name="aT", bufs=1))
        ep = ctx.enter_context(tc.tile_pool(name="ep", bufs=2))
        pT = ctx.enter_context(tc.tile_pool(name="pT", bufs=2, space="PSUM"))
        pZ = ctx.enter_context(tc.tile_pool(name="pZ", bufs=2, space="PSUM"))

        xs = big.tile([P, J * F], f32)
        wb = big.tile([P, SD], f32)
        wt = big.tile([P, SD], f32)
        idxs = big.tile([P, SD], i32)
        deg = big.tile([P, J], f32)
        rec = big.tile([P, J], f32)
        dinv = big.tile([P, J], f32)
        b1t = big.tile([P, F], f32)
        b2t = big.tile([P, F], f32)
        W1t = big.tile([F, F], f32)
        W2t = big.tile([F, F], f32)
        ident = big.tile([P, P], f32)
        agg = big.tile([P, J * F], f32)
        zb = big.tile([P, J * F], f32)

        # ---- loads ----
        nc.sync.dma_start(out=xs[:], in_=x_in[:, :])
        nc.sync.dma_start(out=wb[:], in_=w_in[:, :])
        nc.sync.dma_start(out=idxs[:], in_=idx_in[:, :])
        nc.sync.dma_start(out=W1t[:], in_=W1_in[:, :])
        nc.sync.dma_start(out=W2t[:], in_=W2_in[:, :])
        nc.sync.dma_start(out=b1t[:], in_=b1_in[:, :])
        nc.sync.dma_start(out=b2t[:], in_=b2_in[:, :])
        make_identity(nc, ident[:])

        # ---- degrees / dinv / w~ ----
        # deg = sum of in-edge weights + 1 (the self-loop, handled separately)
        nc.vector.memset(deg[:], 0.0)
        for j in range(J):
            if off[j + 1] > off[j]:
                nc.vector.reduce_sum(
                    out=deg[:, j:j + 1],
                    in_=wb[:, int(off[j]):int(off[j + 1])],
                    axis=mybir.AxisListType.X,
                )
        nc.vector.tensor_scalar_add(out=rec[:], in0=deg[:], scalar1=1.0)
        nc.vector.reciprocal(deg[:], rec[:])
        nc.scalar.sqrt(dinv[:], deg[:])
        for j in range(J):
            if off[j + 1] > off[j]:
                nc.vector.tensor_scalar_mul(
                    out=wt[:, int(off[j]):int(off[j + 1])],
                    in0=wb[:, int(off[j]):int(off[j + 1])],
                    scalar1=dinv[:, j:j + 1],
                )

        # ---- x' = dinv * x -> ag_in1 -> AllGather -> table1 ----
        nc.vector.tensor_tensor(
            out=zb[:].rearrange("p (j f) -> p j f", f=F),
            in0=xs[:].rearrange("p (j f) -> p j f", f=F),
            in1=dinv[:].unsqueeze(2).to_broadcast([P, J, F]),
            op=mybir.AluOpType.mult,
        )
        ag1_ap = ag1.ap().rearrange("(p j) f -> p (j f)", p=P)
        nc.sync.dma_start(out=ag1_ap, in_=zb[:])
        nc.gpsimd.collective_compute(
            "AllGather", mybir.AluOpType.bypass, replica_groups=groups,
            ins=[ag1.ap().opt()], outs=[table1.ap().opt()],
        )

        def aggregate(table):
            # HW indirect DMA honors one offset per partition per instruction
            # (the [P, 1] pattern), so gather one slot-column (128 rows) at a
            # time.  Empty (zero-degree) groups keep their memset slice.
            nc.vector.memset(agg[:], 0.0)
            for (j0, j1, o0, o1) in batches:
                S = o1 - o0
                g = gp.tile([P, maxS * F], f32, tag="g")
                for d in range(S):
                    nc.gpsimd.indirect_dma_start(
                        out=g[:, d * F:(d + 1) * F],
                        out_offset=None,
                        in_=table[:, :],
                        in_offset=bass.IndirectOffsetOnAxis(
                            ap=idxs[:, o0 + d:o0 + d + 1], axis=0),
                    )
                nc.vector.tensor_tensor(
                    out=g[:, :S * F].rearrange("p (s f) -> p s f", f=F),
                    in0=g[:, :S * F].rearrange("p (s f) -> p s f", f=F),
                    in1=wt[:, o0:o1].unsqueeze(2).to_broadcast([P, S, F]),
                    op=mybir.AluOpType.mult,
                )
                for j in range(j0, j1):
                    D = int(Dhat[j])
                    if D == 0:
                        continue
                    rel = int(off[j]) - o0
                    mj = g[:, rel * F:(rel + D) * F].rearrange(
                        "p (d f) -> p f d", f=F)
                    nc.vector.reduce_sum(
                        out=agg[:, j * F:(j + 1) * F],
                        in_=mj,
                        axis=mybir.AxisListType.X,
                    )

        def transform(Wt, bt, scale_dinv):
            for w0 in range(0, J, WAVE):
                w1 = min(w0 + WAVE, J)
                nW = w1 - w0
                # matmul input = agg + dinv * zb   (self-loop contribution:
                # zb holds this layer's dinv-prescaled input rows)
                tsf = ep.tile([P, WAVE * F], f32, tag="sf")
                nc.vector.tensor_tensor(
                    out=tsf[:, :nW * F].rearrange("p (j f) -> p j f", f=F),
                    in0=zb[:, w0 * F:w1 * F].rearrange("p (j f) -> p j f", f=F),
                    in1=dinv[:, w0:w1].unsqueeze(2).to_broadcast([P, nW, F]),
                    op=mybir.AluOpType.mult,
                )
                tsum = ep.tile([P, WAVE * F], f32, tag="ts")
                nc.vector.tensor_tensor(
                    out=tsum[:, :nW * F],
                    in0=tsf[:, :nW * F],
                    in1=agg[:, w0 * F:w1 * F],
                    op=mybir.AluOpType.add,
                )
                aggT = aT.tile([F, WAVE * P], f32, tag="aT")
                nhalf = math.ceil(nW / 4)
                for h in range(nhalf):
                    lo = w0 + h * 4
                    hi = min(lo + 4, w1)
                    psT = pT.tile([F, 4 * P], f32, tag="pT")
                    for i, j in enumerate(range(lo, hi)):
                        jj = j - w0
                        nc.tensor.transpose(
                            out=psT[:, i * P:(i + 1) * P],
                            in_=tsum[:, jj * F:(jj + 1) * F],
                            identity=ident[:],
                        )
                    nn = hi - lo
                    nc.vector.tensor_copy(
                        out=aggT[:, (h * 4) * P:(h * 4 + nn) * P],
                        in_=psT[:, :nn * P],
                    )
                psZ = pZ.tile([P, WAVE * F], f32, tag="pZ")
                for i, j in enumerate(range(w0, w1)):
                    nc.tensor.matmul(
                        out=psZ[:, i * F:(i + 1) * F],
                        lhsT=aggT[:, i * P:(i + 1) * P],
                        rhs=Wt[:],
                        start=True, stop=True,
                    )
                e1 = ep.tile([P, WAVE * F], f32, tag="e1")
                nc.vector.tensor_tensor(
                    out=e1[:, :nW * F].rearrange("p (j f) -> p j f", f=F),
                    in0=psZ[:, :nW * F].rearrange("p (j f) -> p j f", f=F),
                    in1=bt[:].unsqueeze(1).to_broadcast([P, nW, F]),
                    op=mybir.AluOpType.add,
                )
                if scale_dinv:
                    e2 = ep.tile([P, WAVE * F], f32, tag="e2")
                    nc.vector.tensor_tensor(
                        out=e2[:, :nW * F].rearrange("p (j f) -> p j f", f=F),
                        in0=e1[:, :nW * F].rearrange("p (j f) -> p j f", f=F),
                        in1=dinv[:, w0:w1].unsqueeze(2).to_broadcast([P, nW, F]),
                        op=mybir.AluOpType.mult,
                    )
                    src = e2
                else:
                    src = e1
                nc.scalar.activation(
                    out=zb[:, w0 * F:w1 * F],
                    in_=src[:, :nW * F],
                    func=mybir.ActivationFunctionType.Relu,
                )

        # ---- layer 1 ----
        with nc.named_scope("agg1"):
            aggregate(table1)
        with nc.named_scope("xform1"):
            transform(W1t, b1t, scale_dinv=True)
        with nc.named_scope("allgather2"):
            ag2_ap = ag2.ap().rearrange("(p j) f -> p (j f)", p=P)
            nc.sync.dma_start(out=ag2_ap, in_=zb[:])
            nc.gpsimd.collective_compute(
                "AllGather", mybir.AluOpType.bypass, replica_groups=groups,
                ins=[ag2.ap().opt()], outs=[table2.ap().opt()],
            )

        # ---- layer 2 ----
        with nc.named_scope("agg2"):
            aggregate(table2)
        with nc.named_scope("xform2"):
            transform(W2t, b2t, scale_dinv=False)
        nc.sync.dma_start(out=out_t[:, :], in_=zb[:])

    nc.compile()
    return nc


# ---------------------------------------------------------------------------
# Entry point
# ---------------------------------------------------------------------------

def _make_in_maps(plan, node_feats, W1, b1, W2, b2):
    x_slices = _shard_x(node_feats, plan)
    W1 = np.ascontiguousarray(np.asarray(W1, np.float32))
    W2 = np.ascontiguousarray(np.asarray(W2, np.float32))
    b1t = np.ascontiguousarray(np.broadcast_to(
        np.asarray(b1, np.float32)[None, :], (P, F)))
    b2t = np.ascontiguousarray(np.broadcast_to(
        np.asarray(b2, np.float32)[None, :], (P, F)))
    in_maps = []
    for k in range(C):
        in_maps.append({
            "x_slice": np.ascontiguousarray(x_slices[k]),
            "w_pad": np.ascontiguousarray(plan["w_pad"][k]),
            "idx": np.ascontiguousarray(plan["idx"][k]),
            "W1": W1, "W2": W2, "b1": b1t, "b2": b2t,
        })
    return in_maps


def _unshard(plan, outs):
    J, N = plan["J"], plan["N"]
    full = np.concatenate(
        [o.reshape(P, J, F).reshape(P * J, F) for o in outs], axis=0)
    return np.ascontiguousarray(full[plan["t_of"][:N]])


LAST_RESULT = None  # BassKernelResults of the most recent kernel() call


def kernel(node_feats, edge_index, edge_feats, W1, b1, W2, b2):
    global LAST_RESULT
    from concourse.bass_utils import run_bass_kernel_spmd

    plan = _plan(node_feats.shape[0], edge_index, edge_feats)
    nc = _build(plan)
    in_maps = _make_in_maps(plan, node_feats, W1, b1, W2, b2)
    res = run_bass_kernel_spmd(nc, in_maps, core_ids=list(range(C)))
    LAST_RESULT = res
    return _unshard(plan, [res.results[k]["out"] for k in range(C)])



# revision 13
# speedup vs baseline: 1.2580x; 1.2580x over previous
"""Two-layer GCN (PyG GCNConv semantics) on 8 Trainium2 NeuronCores.

Strategy (1D graph partitioning, destination-sharded, SWDGE dma_gather):
  * Nodes are sorted by in-degree (descending), padded to a multiple of
    128*8, and chunked into groups of 128.  Group g is owned by core g%8.
    Node identity on device = "table row" t = k*(J*128) + p*J + j for core
    k, partition slot p, local group j.
  * The gather table holds fp16 feature rows in t-order, viewed as QUADS:
    table row q = nodes 4q..4q+3 (512 B).  Quad count 25088 fits the int16
    index range of the SWDGE dma_gather instruction, so each gather batch
    is ONE instruction covering thousands of edge slots (the baseline
    issued one indirect DMA per 128 rows and serialized ~1us/instruction
    on the Pool engine).  The sub-quad position r = t%4 of each source is
    resolved by a 4x-wide weight table with host-placed zeros: gathered
    quad slot s contributes w4[4s+r] * table[4q+r] and the per-group DVE
    reduce sums over both the slot and sub-quad axes.
  * Per-edge index work happens on the host: each destination node gets
    Dhat_j padded edge slots; padding slots have w4=0 and point at quad 0.
  * dinv[src] is folded into the table (x' = dinv * x); dinv[dst] into the
    weights.  Self contribution dinv^2 * x is added from the SBUF-resident
    slice.  Aggregation runs before the 64x64 weight matmul.
  * Layer-1 table is built LOCALLY on every core from the full permuted
    node features (an ExternalInput) after a tiny [N] AllGather of dinv,
    so no 26 MB feature AllGather is needed.  Layer-2 table comes from one
    fp16 AllGather of the layer-1 output.
"""

import math
import sys
from contextlib import ExitStack

import numpy as np

if "/opt/trn_rl_repo" not in sys.path:
    sys.path.insert(0, "/opt/trn_rl_repo")

P = 128  # SBUF partitions
C = 8    # NeuronCores
F = 64   # feature width (in = hidden = out = 64)
SB = 8           # max quad-slots per gather batch (HW caps dma_gather at
                 # num_idxs <= 1024 = SB*128 per instruction)
WAVE = 8         # groups per transform wave (8*64 = 512 = one PSUM bank)


# ---------------------------------------------------------------------------
# Host-side graph preprocessing (integer index work + permutations only)
# ---------------------------------------------------------------------------

def _plan(n_nodes, edge_index, edge_feats):
    N = int(n_nodes)
    G0 = math.ceil(N / P)
    G_total = math.ceil(G0 / C) * C
    J = G_total // C
    N_pad = G_total * P
    JP = P * J
    Q = N_pad // 4

    row = np.asarray(edge_index[0], dtype=np.int64)
    col = np.asarray(edge_index[1], dtype=np.int64)
    w = np.asarray(edge_feats, dtype=np.float32)

    degc = np.bincount(col, minlength=N_pad)    # real in-degree (may be 0)
    order = np.argsort(-degc, kind="stable")    # descending degree
    s_of = np.empty(N_pad, np.int64)
    s_of[order] = np.arange(N_pad)
    g_of = s_of // P
    p_of = s_of % P
    k_of = g_of % C
    j_of = g_of // C
    t_of = k_of * JP + p_of * J + j_of          # table row per node

    # per-group max degree; descending order => stripe max is the first one
    Dg = degc[order[np.arange(G_total) * P]]
    Dhat = Dg[0::C].astype(np.int64)            # [J]
    off = np.concatenate([[0], np.cumsum(Dhat)]).astype(np.int64)
    SD = int(off[-1])

    # edge slot assignment: sort edges by (dest table row, src table row) —
    # the src minor key clusters gather descriptors by HBM address
    tdst = t_of[col]
    tsrc_all = t_of[row]
    oE = np.lexsort((tsrc_all, tdst))
    td = tdst[oE]
    dslot = np.arange(len(td), dtype=np.int64) - np.searchsorted(td, td, side="left")
    kk = td // JP
    rem = td - kk * JP
    pp = rem // J
    jj = rem - pp * J
    assert np.all(dslot < Dhat[jj]), "edge slot exceeded padded degree"
    colpos = off[jj] + dslot                    # slot s of each edge

    tsrc = tsrc_all[oE]
    qq = (tsrc // 4).astype(np.int16)
    rr = (tsrc % 4).astype(np.int64)

    w4 = np.zeros((C, P, SD * 4), np.float32)
    w4[kk, pp, colpos * 4 + rr] = w[oE]

    # int16 quad indices, wrapped: gathered row i=(s_rel*128+p) reads the idx
    # at [i%16, i//16]; per-slot that is [p%16, s*8 + p//16].  The 8 Q7 cores
    # of the Pool engine each read their own 16-partition stripe -> replicate.
    idx16 = np.zeros((C, 16, SD * 8), np.int16)
    idx16[kk, pp % 16, colpos * 8 + pp // 16] = qq
    idx16 = np.tile(idx16, (1, 8, 1))           # [C, 128, SD*8]

    # gather batches: slot ranges of <= SB columns.  Groups larger than SB
    # split across batches; each batch carries its reduce segments
    # (j, a, b, first) with absolute slot columns [a, b) of group j.
    batches = []  # (o0, o1, segs)
    o0 = 0
    while o0 < SD:
        hi = min(o0 + SB, SD)
        # snap down to the last group boundary in (o0, hi] when one exists
        jr = int(np.searchsorted(off, hi, side="right")) - 1
        o1 = int(off[jr]) if off[jr] > o0 else hi
        segs = []
        j = int(np.searchsorted(off, o0, side="right")) - 1
        while j < J and off[j] < o1:
            a = max(int(off[j]), o0)
            b = min(int(off[j + 1]), o1)
            if b > a:
                segs.append((j, a, b, a == int(off[j])))
            j += 1
        batches.append((o0, o1, segs))
        o0 = o1

    return dict(N=N, N_pad=N_pad, J=J, JP=JP, Q=Q, SD=SD, Dhat=Dhat, off=off,
                t_of=t_of, w4=w4, idx16=idx16, batches=batches)


def _shard_x(node_feats, plan):
    N, N_pad, J = plan["N"], plan["N_pad"], plan["J"]
    x_perm = np.zeros((N_pad, F), np.float32)
    x_perm[plan["t_of"][:N]] = np.asarray(node_feats, dtype=np.float32)
    xc = x_perm.reshape(C, P, J, F)
    x_full = np.ascontiguousarray(xc.transpose(1, 0, 2, 3).reshape(P, C * J * F))
    x_slices = xc.reshape(C, P, J * F)
    return x_full, x_slices


# ---------------------------------------------------------------------------
# Device program
# ---------------------------------------------------------------------------

def _build(plan):
    from concourse import bacc, bass, mybir
    import concourse.tile as tile
    from concourse.masks import make_identity

    f32 = mybir.dt.float32
    f16 = mybir.dt.float16
    i16 = mybir.dt.int16
    J, SD, JP, Q = plan["J"], plan["SD"], plan["JP"], plan["Q"]
    Dhat, off, batches = plan["Dhat"], plan["off"], plan["batches"]
    N_pad = plan["N_pad"]
    maxS = max(o1 - o0 for (o0, o1, _) in batches)

    nc = bacc.Bacc(None, target_bir_lowering=False, num_devices=C)

    xf_in = nc.dram_tensor("x_full", [P, C * J * F], f32, kind="ExternalInput")
    xs_in = nc.dram_tensor("x_slice", [P, J * F], f32, kind="ExternalInput")
    w4_in = nc.dram_tensor("w4", [P, SD * 4], f32, kind="ExternalInput")
    idx_in = nc.dram_tensor("idx16", [P, SD * 8], i16, kind="ExternalInput")
    W1_in = nc.dram_tensor("W1", [F, F], f32, kind="ExternalInput")
    W2_in = nc.dram_tensor("W2", [F, F], f32, kind="ExternalInput")
    b1_in = nc.dram_tensor("b1", [P, F], f32, kind="ExternalInput")
    b2_in = nc.dram_tensor("b2", [P, F], f32, kind="ExternalInput")
    out_t = nc.dram_tensor("out", [P, J * F], f32, kind="ExternalOutput")

    dinv_d = nc.dram_tensor("dinv_own", [JP], f32)
    dinv_all = nc.dram_tensor("dinv_all", [C * JP], f32)
    table1 = nc.dram_tensor("table1", [N_pad, F], f16)
    ag2 = nc.dram_tensor("ag_in2", [JP, F], f16)
    table2 = nc.dram_tensor("table2", [C * JP, F], f16)

    groups = [list(range(C))]
    # table1 build chunks must be whole core blocks: rows k*P*J..(k+1)*P*J are
    # contiguous in t-order, sub-core splits are not.
    xchunk = C
    JC = J                          # groups per build chunk
    CW = JC * F                     # columns per build chunk

    with ExitStack() as ctx:
        tc = ctx.enter_context(tile.TileContext(nc))
        big = ctx.enter_context(tc.tile_pool(name="big", bufs=1))
        aT = ctx.enter_context(tc.tile_pool(name="aT", bufs=1))
        ep = ctx.enter_context(tc.tile_pool(name="ep", bufs=2))
        pT = ctx.enter_context(tc.tile_pool(name="pT", bufs=2, space="PSUM"))
        pZ = ctx.enter_context(tc.tile_pool(name="pZ", bufs=2, space="PSUM"))

        wt4h = big.tile([P, SD * 4], f16)
        idxs = big.tile([P, SD * 8], i16)
        deg = big.tile([P, J], f32)
        rec = big.tile([P, J], f32)
        dinv = big.tile([P, J], f32)
        dinva = big.tile([P, C * J], f32)
        b1t = big.tile([P, F], f32)
        b2t = big.tile([P, F], f32)
        W1t = big.tile([F, F], f32)
        W2t = big.tile([F, F], f32)
        ident = big.tile([P, P], f32)
        agg = big.tile([P, J * F], f32)
        zb = big.tile([P, J * F], f32)

        # ---- setup: loads, degrees, dinv, weights, own prescale ----
        with nc.named_scope("setup"), tc.tile_pool(name="sp", bufs=1) as sp:
            xs = sp.tile([P, J * F], f32)
            w4t = sp.tile([P, SD * 4], f32)
            nc.sync.dma_start(out=w4t[:], in_=w4_in[:, :])
            nc.sync.dma_start(out=idxs[:], in_=idx_in[:, :])
            nc.sync.dma_start(out=xs[:], in_=xs_in[:, :])
            nc.sync.dma_start(out=W1t[:], in_=W1_in[:, :])
            nc.sync.dma_start(out=W2t[:], in_=W2_in[:, :])
            nc.sync.dma_start(out=b1t[:], in_=b1_in[:, :])
            nc.sync.dma_start(out=b2t[:], in_=b2_in[:, :])
            make_identity(nc, ident[:])

            # deg = sum of in-edge weights (+1 for the self-loop below)
            nc.vector.memset(deg[:], 0.0)
            for j in range(J):
                if off[j + 1] > off[j]:
                    nc.vector.reduce_sum(
                        out=deg[:, j:j + 1],
                        in_=w4t[:, 4 * int(off[j]):4 * int(off[j + 1])],
                        axis=mybir.AxisListType.X,
                    )
            nc.vector.tensor_scalar_add(out=rec[:], in0=deg[:], scalar1=1.0)
            nc.vector.reciprocal(deg[:], rec[:])
            nc.scalar.sqrt(dinv[:], deg[:])
            # w~ = w * dinv[dst]  (fp16, 4x-wide with sub-quad zeros)
            for j in range(J):
                if off[j + 1] > off[j]:
                    nc.vector.tensor_scalar_mul(
                        out=wt4h[:, 4 * int(off[j]):4 * int(off[j + 1])],
                        in0=w4t[:, 4 * int(off[j]):4 * int(off[j + 1])],
                        scalar1=dinv[:, j:j + 1],
                    )
            # zb = dinv * x  (own slice, f32; self-loop source for layer 1)
            nc.vector.tensor_tensor(
                out=zb[:].rearrange("p (j f) -> p j f", f=F),
                in0=xs[:].rearrange("p (j f) -> p j f", f=F),
                in1=dinv[:].unsqueeze(2).to_broadcast([P, J, F]),
                op=mybir.AluOpType.mult,
            )
            # AllGather dinv (tiny) so every core can build table1 locally
            nc.sync.dma_start(out=dinv_d.ap().rearrange("(p j) -> p j", p=P),
                              in_=dinv[:])
            nc.gpsimd.collective_compute(
                "AllGather", mybir.AluOpType.bypass, replica_groups=groups,
                ins=[dinv_d.ap().opt()], outs=[dinv_all.ap().opt()],
            )
            nc.sync.dma_start(
                out=dinva[:].rearrange("p (k j) -> p k j", j=J),
                in_=dinv_all.ap().rearrange("(k p j) -> p k j", p=P, j=J),
            )

        # ---- table1 = fp16(dinv_all * x_full), built locally per core ----
        with nc.named_scope("table1"), tc.tile_pool(name="cp", bufs=2) as cp:
            for k in range(xchunk):
                xk = cp.tile([P, CW], f32, tag="xk")
                nc.sync.dma_start(out=xk[:], in_=xf_in[:, k * CW:(k + 1) * CW])
                zk = cp.tile([P, CW], f16, tag="zk")
                nc.vector.tensor_tensor(
                    out=zk[:].rearrange("p (j f) -> p j f", f=F),
                    in0=xk[:].rearrange("p (j f) -> p j f", f=F),
                    in1=dinva[:, k * JC:(k + 1) * JC].unsqueeze(2)
                        .to_broadcast([P, JC, F]),
                    op=mybir.AluOpType.mult,
                )
                nc.sync.dma_start(
                    out=table1.ap()[k * P * JC:(k + 1) * P * JC]
                        .rearrange("(p j) f -> p (j f)", p=P),
                    in_=zk[:],
                )

        gp = ctx.enter_context(tc.tile_pool(name="gp", bufs=4))

        def aggregate(table):
            # table viewed as quads [Q, 256]; one dma_gather per batch.
            tq = table.ap().rearrange("(q g) f -> q (g f)", g=4)
            nc.vector.memset(agg[:], 0.0)
            for (o0, o1, segs) in batches:
                S = o1 - o0
                g = gp.tile([P, maxS * 256], f16, tag="g")
                nc.gpsimd.dma_gather(
                    out_ap=g[:, :S * 256].rearrange("p (s g) -> p s g", g=256),
                    in_ap=tq,
                    idxs_ap=idxs[:, 8 * o0:8 * o1],
                    num_idxs=S * P,
                    num_idxs_reg=S * P,
                    elem_size=256,
                )
                nc.vector.tensor_tensor(
                    out=g[:, :S * 256].rearrange("p (s f) -> p s f", f=F),
                    in0=g[:, :S * 256].rearrange("p (s f) -> p s f", f=F),
                    in1=wt4h[:, 4 * o0:4 * o1].unsqueeze(2)
                        .to_broadcast([P, 4 * S, F]),
                    op=mybir.AluOpType.mult,
                )
                for (j, a, b, first) in segs:
                    mj = g[:, (a - o0) * 256:(b - o0) * 256].rearrange(
                        "p (s f) -> p f s", f=F)
                    if first:
                        nc.vector.reduce_sum(
                            out=agg[:, j * F:(j + 1) * F],
                            in_=mj,
                            axis=mybir.AxisListType.X,
                        )
                    else:
                        tmp = ep.tile([P, F], f32, tag="tmpr")
                        nc.vector.reduce_sum(
                            out=tmp[:], in_=mj, axis=mybir.AxisListType.X)
                        nc.vector.tensor_tensor(
                            out=agg[:, j * F:(j + 1) * F],
                            in0=agg[:, j * F:(j + 1) * F],
                            in1=tmp[:],
                            op=mybir.AluOpType.add,
                        )

        def transform(Wt, bt, scale_dinv, cast16_to=None):
            for w0 in range(0, J, WAVE):
                w1 = min(w0 + WAVE, J)
                nW = w1 - w0
                # matmul input = agg + dinv * zb   (self-loop contribution)
                tsf = ep.tile([P, WAVE * F], f32, tag="sf")
                nc.vector.tensor_tensor(
                    out=tsf[:, :nW * F].rearrange("p (j f) -> p j f", f=F),
                    in0=zb[:, w0 * F:w1 * F].rearrange("p (j f) -> p j f", f=F),
                    in1=dinv[:, w0:w1].unsqueeze(2).to_broadcast([P, nW, F]),
                    op=mybir.AluOpType.mult,
                )
                tsum = ep.tile([P, WAVE * F], f32, tag="ts")
                nc.vector.tensor_tensor(
                    out=tsum[:, :nW * F],
                    in0=tsf[:, :nW * F],
                    in1=agg[:, w0 * F:w1 * F],
                    op=mybir.AluOpType.add,
                )
                aggT = aT.tile([F, WAVE * P], f32, tag="aT")
                nhalf = math.ceil(nW / 4)
                for h in range(nhalf):
                    lo = w0 + h * 4
                    hi = min(lo + 4, w1)
                    psT = pT.tile([F, 4 * P], f32, tag="pT")
                    for i, j in enumerate(range(lo, hi)):
                        jj = j - w0
                        nc.tensor.transpose(
                            out=psT[:, i * P:(i + 1) * P],
                            in_=tsum[:, jj * F:(jj + 1) * F],
                            identity=ident[:],
                        )
                    nn = hi - lo
                    nc.vector.tensor_copy(
                        out=aggT[:, (h * 4) * P:(h * 4 + nn) * P],
                        in_=psT[:, :nn * P],
                    )
                psZ = pZ.tile([P, WAVE * F], f32, tag="pZ")
                for i, j in enumerate(range(w0, w1)):
                    nc.tensor.matmul(
                        out=psZ[:, i * F:(i + 1) * F],
                        lhsT=aggT[:, i * P:(i + 1) * P],
                        rhs=Wt[:],
                        start=True, stop=True,
                    )
                e1 = ep.tile([P, WAVE * F], f32, tag="e1")
                nc.vector.tensor_tensor(
                    out=e1[:, :nW * F].rearrange("p (j f) -> p j f", f=F),
                    in0=psZ[:, :nW * F].rearrange("p (j f) -> p j f", f=F),
                    in1=bt[:].unsqueeze(1).to_broadcast([P, nW, F]),
                    op=mybir.AluOpType.add,
                )
                if scale_dinv:
                    e2 = ep.tile([P, WAVE * F], f32, tag="e2")
                    nc.vector.tensor_tensor(
                        out=e2[:, :nW * F].rearrange("p (j f) -> p j f", f=F),
                        in0=e1[:, :nW * F].rearrange("p (j f) -> p j f", f=F),
                        in1=dinv[:, w0:w1].unsqueeze(2).to_broadcast([P, nW, F]),
                        op=mybir.AluOpType.mult,
                    )
                    src = e2
                else:
                    src = e1
                nc.scalar.activation(
                    out=zb[:, w0 * F:w1 * F],
                    in_=src[:, :nW * F],
                    func=mybir.ActivationFunctionType.Relu,
                )
                if cast16_to is not None:
                    h16 = ep.tile([P, WAVE * F], f16, tag="h16")
                    nc.vector.tensor_copy(
                        out=h16[:, :nW * F], in_=zb[:, w0 * F:w1 * F])
                    nc.sync.dma_start(
                        out=cast16_to[:, w0 * F:w1 * F], in_=h16[:, :nW * F])

        # ---- layer 1 ----
        ag2_ap = ag2.ap().rearrange("(p j) f -> p (j f)", p=P)
        with nc.named_scope("agg1"):
            aggregate(table1)
        with nc.named_scope("xform1"):
            transform(W1t, b1t, scale_dinv=True, cast16_to=ag2_ap)
        with nc.named_scope("allgather2"):
            nc.gpsimd.collective_compute(
                "AllGather", mybir.AluOpType.bypass, replica_groups=groups,
                ins=[ag2.ap().opt()], outs=[table2.ap().opt()],
            )

        # ---- layer 2 ----
        with nc.named_scope("agg2"):
            aggregate(table2)
        with nc.named_scope("xform2"):
            transform(W2t, b2t, scale_dinv=False)
        nc.sync.dma_start(out=out_t[:, :], in_=zb[:])

    nc.compile()
    return nc


# ---------------------------------------------------------------------------
# Entry point
# ---------------------------------------------------------------------------

def _make_in_maps(plan, node_feats, W1, b1, W2, b2):
    x_full, x_slices = _shard_x(node_feats, plan)
    W1 = np.ascontiguousarray(np.asarray(W1, np.float32))
    W2 = np.ascontiguousarray(np.asarray(W2, np.float32))
    b1t = np.ascontiguousarray(np.broadcast_to(
        np.asarray(b1, np.float32)[None, :], (P, F)))
    b2t = np.ascontiguousarray(np.broadcast_to(
        np.asarray(b2, np.float32)[None, :], (P, F)))
    in_maps = []
    for k in range(C):
        in_maps.append({
            "x_full": x_full,
            "x_slice": np.ascontiguousarray(x_slices[k]),
            "w4": np.ascontiguousarray(plan["w4"][k]),
            "idx16": np.ascontiguousarray(plan["idx16"][k]),
            "W1": W1, "W2": W2, "b1": b1t, "b2": b2t,
        })
    return in_maps


def _unshard(plan, outs):
    J, N = plan["J"], plan["N"]
    full = np.concatenate(
        [o.reshape(P, J, F).reshape(P * J, F) for o in outs], axis=0)
    return np.ascontiguousarray(full[plan["t_of"][:N]])


LAST_RESULT = None  # BassKernelResults of the most recent kernel() call


def kernel(node_feats, edge_index, edge_feats, W1, b1, W2, b2):
    global LAST_RESULT
    from concourse.bass_utils import run_bass_kernel_spmd

    plan = _plan(node_feats.shape[0], edge_index, edge_feats)
    nc = _build(plan)
    in_maps = _make_in_maps(plan, node_feats, W1, b1, W2, b2)
    res = run_bass_kernel_spmd(nc, in_maps, core_ids=list(range(C)))
    LAST_RESULT = res
    return _unshard(plan, [res.results[k]["out"] for k in range(C)])
